# revision 53
# baseline (speedup 1.0000x reference)
"""DiT block (adaLN) Trainium2 kernel, 8-core SPMD, no collectives.

Sharding: core c handles batch b = c//2 and query-token half c%2 (1024 q
tokens).  Each core computes K/V for all 2048 tokens of its batch (the
only duplicated work), so cores never communicate.  The host permutes
each core's token columns so its own 1024 tokens come first (softmax is
invariant to key order), and transposes x to feature-major [D, L] so the
device never transposes anything.

On-device layout is feature-major everywhere: activations live as
[128 partitions, d-chunk, tokens].  LayerNorm stats (per-token = free
dim) are computed with ones-vector matmuls on the tensor engine (bf16)
and broadcast back across partitions on GpSimd.  All GEMM operands are
bf16 (fp32 PSUM accumulation); the residual stream and LN statistics
stay fp32.

Perf changes vs the 1.95ms baseline (measured 1.08ms):
 - all weight DMAs batched into multi-tile band/block transfers (the SP
   queue pays ~565ns per dma_start regardless of size; the baseline's
   1205 single-tile loads cost ~700us of queue serialization).
 - time-modulation matvec is silu-stationary (free-dim-512 matmuls over
   Wt bands) + 48 tiny transpose-matmuls, instead of 384 free-dim-1
   weight-stationary matmuls each with its own 32KB DMA.
 - LN stats in bf16 (fp32 matmuls run at 1/4 PE rate); the other token
   half arrives host-pre-cast to bf16, skipping the Scalar casts; LN apply
   split across DVE (mult), GpSimd (sub) and Scalar (modulate).
 - phase A interleaves LN1, the tp matvec and the Q/K rounds so the PE
   stays fed while the LN chains drain; Q's output buffer doubles as the
   attention output and the MLP's gelu/h2 buffers alias dead K/V/Q
   storage, fitting everything in SBUF without extra allocations.
 - softmax reciprocal on DVE reciprocal_approx_fast (~5x faster than
   reciprocal; the DVE RECIPROCAL op measured 3.3us per 512 tokens).
 - attention output written straight into SBUF (even heads by the DVE,
   odd heads via one 64-partition-offset SBUF->SBUF DMA per tile)
   instead of a DRAM round trip.
"""

import os
import sys
from contextlib import ExitStack

os.environ.setdefault("MYCRO_LOCAL_CACHE", "1")
for _p in ("/opt/trn_rl_repo", "/root/.axon_site/_ro/trn_rl_repo"):
    if os.path.isdir(_p) and _p not in sys.path:
        sys.path.insert(0, _p)

import ml_dtypes
import numpy as np

import concourse.bass as bass
import concourse.tile as tile
from concourse import bacc, mybir
from concourse.bass_utils import run_bass_kernel_spmd

B, L, D, H, HD, MLPD = 4, 2048, 1024, 16, 64, 4096
NCORES = 8
LOWN = L // 2          # own query tokens per core
DC = D // 128          # 8 chunks of the model dim
MC = MLPD // 128       # 32 chunks of the mlp dim
LT = 512               # token tile for matmul free dim
NLT_OWN = LOWN // LT   # 2 token tiles (queries)

f32 = mybir.dt.float32
bf16 = mybir.dt.bfloat16
AF = mybir.ActivationFunctionType
ALU = mybir.AluOpType
BF = ml_dtypes.bfloat16

PACK_QK = True
DEBUG = bool(int(os.environ.get("KERNEL_DEBUG", "0")))


def build_program():
    # Bacc (not plain Bass): its compile() pass legalizes multi-semaphore
    # waits (event semaphores, nop fusion) that walrus can't encode raw.
    nc = bacc.Bacc()

    def _in(name, shape, dtype):
        return nc.declare_dram_parameter(name, shape, dtype, False)[:]

    xfm = _in("xfm", [D, LOWN], f32)
    xoth = _in("xoth_bf", [D, LOWN], bf16)
    temb = _in("temb", [128, DC], f32)
    wqkv = _in("wqkv", [D, 3 * D], bf16)   # Q section pre-scaled by 1/8
    bq = _in("bq", [128, DC], f32)         # pre-scaled by 1/8
    bk = _in("bk", [128, DC], f32)
    bv = _in("bv", [1, D], f32)
    wproj = _in("wproj", [D, D], bf16)
    bproj = _in("bproj", [128, DC], f32)
    w1 = _in("w1", [D, MLPD], bf16)
    b1 = _in("b1", [128, MC], f32)
    w2 = _in("w2", [MLPD, D], bf16)
    b2 = _in("b2", [128, DC], f32)
    wt = _in("wt", [D, 6 * D], bf16)
    bt = _in("bt", [128, 48], f32)
    out = nc.declare_dram_parameter("out_fm", [D, LOWN], f32, True)[:]

    dbg = {}
    if DEBUG:
        dbg["tp"] = nc.declare_dram_parameter("dbg_tp", [128, 48], f32,
                                              True)[:]
        dbg["xmod"] = nc.declare_dram_parameter("dbg_xmod", [128, DC, L],
                                                bf16, True)[:]
        dbg["q"] = nc.declare_dram_parameter("dbg_q", [128, DC, LOWN], bf16,
                                             True)[:]
        dbg["k"] = nc.declare_dram_parameter("dbg_k", [128, DC, L], bf16,
                                             True)[:]
        dbg["v"] = nc.declare_dram_parameter("dbg_v",
                                             [128, L // 128, H, HD + 1],
                                             bf16, True)[:]
        dbg["attn"] = nc.declare_dram_parameter("dbg_attn", [128, DC, LOWN],
                                                bf16, True)[:]
        dbg["x1"] = nc.declare_dram_parameter("dbg_x1", [128, DC, LOWN],
                                              f32, True)[:]
        dbg["h2"] = nc.declare_dram_parameter("dbg_h2", [128, DC, LOWN],
                                              bf16, True)[:]

    with tile.TileContext(nc) as tc:
        _emit_kernel(tc, xfm, xoth, temb, wqkv, bq, bk, bv, wproj, bproj,
                     w1, b1, w2, b2, wt, bt, out, dbg)
    nc.finalize()  # runs Bacc.compile(): reg alloc + sync legalization
    return nc


def _emit_kernel(tc, xfm, xoth, temb, wqkv, bq, bk, bv, wproj, bproj, w1, b1,
                 w2, b2, wt, bt, out, dbg=None):
    nc = tc.nc
    dbg = dbg or {}

    # ---- persistent constants / host-prepped vectors (freed last) ----
    ones_bf, fr_ones_bf = tc.tile([128, 1], bf16, name="ones_bf")
    nc.vector.memset(ones_bf, 1.0)
    one1, fr_one1 = tc.tile([1, 1], bf16, name="one1")
    nc.vector.memset(one1, 1.0)
    eps_tile, fr_eps = tc.tile([1, 1], f32, name="eps_tile")
    nc.vector.memset(eps_tile, 1e-5)

    bias_sb = {}
    bias_frees = []
    for name, ap, w in (("bq", bq, DC), ("bk", bk, DC), ("bproj", bproj, DC),
                        ("b1", b1, MC), ("b2", b2, DC), ("bt", bt, 48),
                        ("temb", temb, DC)):
        t, fr = tc.tile([128, w], f32, name=f"sb_{name}")
        nc.sync.dma_start(out=t, in_=ap)
        bias_sb[name] = t
        bias_frees.append(fr)
    bv_bc, fr_bv = tc.tile([128, D], f32, name="bv_bc")
    nc.sync.dma_start(
        out=bv_bc,
        in_=bass.AP(tensor=bv.tensor, offset=bv.offset,
                    ap=[[0, 128]] + [list(x) for x in bv.ap[1:]]))

    # modulation vectors (computed in phase 0, consumed later)
    tp, fr_tp = tc.tile([128, 48], f32, name="tp")
    s_msa, fr_s1 = tc.tile([128, DC], f32, name="s_msa")
    s_mlp, fr_s2 = tc.tile([128, DC], f32, name="s_mlp")
    gmbp, fr_g1 = tc.tile([128, DC], f32, name="gmbp")
    gmb2, fr_g2 = tc.tile([128, DC], f32, name="gmb2")
    shift_msa = tp[:, 0:8]
    gate_msa = tp[:, 16:24]
    shift_mlp = tp[:, 24:32]
    gate_mlp = tp[:, 40:48]

    # ---- big persistent activations, creation order = reverse free order ----
    x_own, fr_x_own = tc.tile([128, DC, LOWN], f32, name="x_own")
    k_sb, fr_k = tc.tile([128, DC, L], bf16, name="k_sb")
    # v_aug: [token-part, token-chunk, head, 65]; col 64 holds ones so the
    # AV matmul also produces the softmax denominator.
    v_aug, fr_v = tc.tile([128, L // 128, H, HD + 1], bf16, name="v_aug")
    q_sb, fr_q = tc.tile([128, DC, LOWN], bf16, name="q_sb")
    xmod, fr_xmod = tc.tile([128, DC, L], bf16, name="xmod")

    xr = xfm.rearrange("(c p) t -> p c t", p=128)
    nc.sync.dma_start(out=x_own, in_=xr)
    xoth_r = xoth.rearrange("(c p) t -> p c t", p=128)

    wt_r = wt.rearrange("(c p) f -> p c f", p=128)        # [128, 8, 6144]
    wqkv_r = wqkv.rearrange("(c p) f -> p c f", p=128)    # [128, 8, 3072]
    wproj_r = wproj.rearrange("(c p) f -> p c f", p=128)  # [128, 8, 1024]
    w1_r = w1.rearrange("(c p) f -> p c f", p=128)        # [128, 8, 4096]
    w2_r = w2.rearrange("(c p) f -> p c f", p=128)        # [128, 32, 1024]

    # ---- LayerNorm helpers (512-token tiles) ------------------------------
    # Stats: per-token sum / sumsq via ones-stationary bf16 matmuls (the
    # fp32 matmul path runs at 1/4 PE rate).  x_bf is an optional bf16 view
    # of the same data (skips the Scalar casts).  Returns (rstd, mua) bf16
    # row tiles [1, LT].
    def ln_stats(sbp, psp, x_view, tag, x_bf=None):
        ps_s = psp.tile([1, LT], f32, tag="st_s", bufs=1, name="ps_s")
        ps_q = psp.tile([1, LT], f32, tag="st_q", bufs=1, name="ps_q")
        for dc in range(DC):
            if x_bf is not None:
                xb = x_bf[:, dc, :]
            else:
                xb = sbp.tile([128, LT], bf16, tag="xb", bufs=1, name="xb")
                nc.scalar.activation(xb, x_view[:, dc, :], AF.Identity)
            nc.tensor.matmul(ps_s, ones_bf, xb,
                             start=(dc == 0), stop=(dc == DC - 1))
            sq = sbp.tile([128, LT], bf16, tag="sq", bufs=1, name="sq")
            nc.vector.tensor_tensor(sq, xb, xb, ALU.mult)
            nc.tensor.matmul(ps_q, ones_bf, sq,
                             start=(dc == 0), stop=(dc == DC - 1))
        mean = sbp.tile([1, LT], f32, tag="ln_mean", bufs=1, name="mean")
        msq = sbp.tile([1, LT], f32, tag="ln_msq", bufs=1, name="msq")
        nc.vector.tensor_scalar_mul(mean, ps_s, 1.0 / D)
        nc.vector.tensor_tensor(msq, mean, mean, ALU.mult)
        # msq <- ps_q/D - mean^2  (= var)
        nc.vector.scalar_tensor_tensor(msq, ps_q, 1.0 / D, msq,
                                       ALU.mult, ALU.subtract)
        sd = sbp.tile([1, LT], f32, tag="ln_sd", bufs=1, name="sd")
        nc.scalar.activation(sd, msq, AF.Sqrt, bias=eps_tile)
        # reciprocal_approx_fast is a custom DVE op: needs partition-0-based
        # input and must not run in place.
        rstd = sbp.tile([1, LT], f32, tag=f"rstd{tag}", bufs=1, name="rstd")
        nc.vector.reciprocal_approx_fast(rstd, sd)
        mua = sbp.tile([1, LT], bf16, tag=f"mua{tag}", bufs=1, name="mua")
        nc.vector.tensor_tensor(mua, mean, rstd, ALU.mult)
        return rstd, mua

    # Apply: out = ((x*rstd_bc - mua_bc) * s_d + sh_d) with the three
    # elementwise passes split DVE / GpSimd / Scalar.
    def ln_apply(sbp, x_view, out_view, rstd, mua, scale_ap, shift_ap,
                 sub_on_pool=True):
        a_bc = sbp.tile([128, LT], f32, tag="a_bc", bufs=1, name="a_bc")
        nc.gpsimd.partition_broadcast(a_bc, rstd)
        m_bc = sbp.tile([128, LT], bf16, tag="m_bc", bufs=1, name="m_bc")
        nc.gpsimd.partition_broadcast(m_bc, mua)
        for dc in range(DC):
            t = sbp.tile([128, LT], bf16, tag="ln_t", bufs=2, name="ln_t")
            nc.vector.tensor_tensor(t, x_view[:, dc, :], a_bc, ALU.mult)
            if sub_on_pool:
                nc.gpsimd.tensor_tensor(t, t, m_bc, ALU.subtract)
            else:
                nc.vector.tensor_tensor(t, t, m_bc, ALU.subtract)
            nc.scalar.activation(
                out_view[:, dc, :], t, AF.Identity,
                bias=shift_ap[:, dc:dc + 1], scale=scale_ap[:, dc:dc + 1])

    # ===== phase A: LN1 + time modulation + Q/K, interleaved for overlap ===
    # Order: own-tile LN stats (PE work gated only on the x_own DMA) ->
    # tp matvec (gated on Wt DMAs) -> own applies -> Q round (PE work that
    # hides the streamed other-half LN chains) -> other-half LN -> K round.
    nc.vector.memset(v_aug[:, :, :, HD:], 1.0)
    with ExitStack() as ph:
        sbp = ph.enter_context(tc.tile_pool(name="pA_sb", bufs=2))
        psp = ph.enter_context(tc.tile_pool(name="pA_ps", bufs=1,
                                            space="PSUM"))

        # silu first so the Sigmoid->Sqrt act-table switch happens once
        sig = sbp.tile([128, DC], f32, tag="sig", bufs=1, name="sig")
        nc.scalar.activation(sig, bias_sb["temb"], AF.Sigmoid)
        silu_bf = sbp.tile([128, DC], bf16, tag="silu", bufs=1, name="silu_bf")
        nc.vector.tensor_tensor(silu_bf, bias_sb["temb"], sig, ALU.mult)

        stats_own = []
        for i in range(NLT_OWN):
            stats_own.append(ln_stats(
                sbp, psp, x_own[:, :, i * LT:(i + 1) * LT], f"o{i}"))

        # --- tp = silu(temb) @ Wt + bt, feature-major [128, 48] ---
        ps_tp = psp.tile([128, 48], f32, tag="tp", bufs=1, name="ps_tp")
        for g in range(6):           # column sixths of Wt (1024 each)
            pss = [psp.tile([1, LT], f32, tag=f"tpr{i}", bufs=1,
                            name=f"ps_tpr{i}") for i in range(2)]
            for dc in range(DC):
                band = sbp.tile([128, 1024], bf16, tag="wtb", bufs=2,
                                name="wtb")
                nc.sync.dma_start(
                    out=band, in_=wt_r[:, dc, g * 1024:(g + 1) * 1024])
                for i in range(2):
                    nc.tensor.matmul(pss[i], silu_bf[:, dc:dc + 1],
                                     band[:, i * LT:(i + 1) * LT],
                                     start=(dc == 0), stop=(dc == DC - 1))
            for i in range(2):
                # [1, 512] row chunk -> 4 transposed [128, 1] PSUM columns
                row = sbp.tile([1, LT], bf16, tag="tprow", bufs=1,
                               name="tprow")
                nc.scalar.activation(row, pss[i], AF.Identity)
                for j in range(4):
                    f = g * 8 + i * 4 + j
                    nc.tensor.matmul(ps_tp[:, f:f + 1],
                                     row[0:1, j * 128:(j + 1) * 128],
                                     one1, start=True, stop=True)
        nc.vector.tensor_tensor(tp, ps_tp, bias_sb["bt"], ALU.add)
        nc.vector.tensor_scalar_add(s_msa, tp[:, 8:16], 1.0)
        nc.vector.tensor_scalar_add(s_mlp, tp[:, 32:40], 1.0)
        nc.vector.tensor_tensor(gmbp, gate_msa, bias_sb["bproj"], ALU.mult)
        nc.vector.tensor_tensor(gmb2, gate_mlp, bias_sb["b2"], ALU.mult)

        # --- LN1 apply for own tiles ---
        for i in range(NLT_OWN):
            rstd, mua = stats_own[i]
            ln_apply(sbp, x_own[:, :, i * LT:(i + 1) * LT],
                     xmod[:, :, i * LT:(i + 1) * LT],
                     rstd, mua, s_msa, shift_msa)

        # --- Q round: 2 column quarters, weight-stationary ---
        def qk_quarter(base_col, f4s, nlt, out_sb, bias_name):
            w4 = sbp.tile([128, DC, 512], bf16, tag="wqk4", bufs=2,
                          name="w4")
            for dc in range(DC):
                nc.sync.dma_start(
                    out=w4[:, dc, :],
                    in_=wqkv_r[:, dc, base_col:base_col + 512])
            for lt0 in range(0, nlt, 2):
                for f4 in range(4):
                    f8 = f4s + f4
                    nl = min(2, nlt - lt0)
                    ps = [psp.tile([128, LT], f32, tag=f"qk{i}", bufs=1,
                                   name=f"ps_qk{i}") for i in range(nl)]
                    for dc in range(DC):
                        for i in range(nl):
                            lt = lt0 + i
                            nc.tensor.matmul(
                                ps[i], w4[:, dc, f4 * 128:(f4 + 1) * 128],
                                xmod[:, dc, lt * LT:(lt + 1) * LT],
                                start=(dc == 0), stop=(dc == DC - 1))
                    for i in range(nl):
                        lt = lt0 + i
                        nc.scalar.activation(
                            out_sb[:, f8, lt * LT:(lt + 1) * LT], ps[i],
                            AF.Identity,
                            bias=bias_sb[bias_name][:, f8:f8 + 1])

        for quarter in range(2):
            qk_quarter(quarter * 512, quarter * 4, NLT_OWN, q_sb, "bq")

        # --- other token half: streamed bf16 tiles (host pre-cast) ---
        for i in range(NLT_OWN):
            xo = sbp.tile([128, DC, LT], bf16, tag="xoth", bufs=1, name="xo")
            nc.sync.dma_start(
                out=xo, in_=xoth_r[:, :, i * LT:(i + 1) * LT])
            rstd, mua = ln_stats(sbp, psp, xo, "s", x_bf=xo)
            ln_apply(sbp, xo,
                     xmod[:, :, LOWN + i * LT:LOWN + (i + 1) * LT],
                     rstd, mua, s_msa, shift_msa)

        # --- K round ---
        for quarter in range(2):
            qk_quarter(1024 + quarter * 512, quarter * 4, L // LT, k_sb,
                       "bk")

        if "tp" in dbg:
            nc.sync.dma_start(out=dbg["tp"], in_=tp)
        if "xmod" in dbg:
            nc.sync.dma_start(out=dbg["xmod"], in_=xmod)

    # ================= phase B: V (x-stationary, token-major) ==============
    with ExitStack() as ph:
        sbp = ph.enter_context(tc.tile_pool(name="pB_sb", bufs=2))
        psp = ph.enter_context(tc.tile_pool(name="pB_ps", bufs=1, space="PSUM"))
        wv_sb = sbp.tile([128, DC, D], bf16, tag="wv", bufs=1, name="wv_sb")
        for dc in range(DC):
            nc.sync.dma_start(out=wv_sb[:, dc, :],
                              in_=wqkv_r[:, dc, 2 * 1024:3 * 1024])
        for tcn in range(L // 128):
            psv = psp.tile([128, 2, LT], f32, tag="v", bufs=2, name="ps_v")
            for dc in range(DC):
                for vs in range(2):
                    nc.tensor.matmul(
                        psv[:, vs, :],
                        xmod[:, dc, tcn * 128:(tcn + 1) * 128],
                        wv_sb[:, dc, vs * LT:(vs + 1) * LT],
                        start=(dc == 0), stop=(dc == DC - 1))
            for vs in range(2):
                nc.vector.tensor_tensor(
                    v_aug[:, tcn, vs * 8:(vs + 1) * 8, :HD],
                    psv[:, vs, :], bv_bc[:, vs * LT:(vs + 1) * LT], ALU.add)
        if "q" in dbg:
            nc.sync.dma_start(out=dbg["q"], in_=q_sb)
            nc.sync.dma_start(out=dbg["k"], in_=k_sb)
            nc.sync.dma_start(out=dbg["v"], in_=v_aug)
    fr_xmod()

    # ================= phase 3: attention ================
    # attn output reuses q_sb's storage: AV for head-chunk hc only writes
    # q columns after the QK matmuls of hc consumed them (WAR deps tracked
    # per-region by the Tile framework).
    attn_sb = q_sb
    with ExitStack() as ph:
        sbp = ph.enter_context(tc.tile_pool(name="p3_sb", bufs=2))
        psp = ph.enter_context(tc.tile_pool(name="p3_ps", bufs=1, space="PSUM"))

        def emit_av(ept_p, hc_p, lt_p):
            lts_p = slice(lt_p * LT, (lt_p + 1) * LT)
            for i in range(2):
                h = 2 * hc_p + i
                ps_av_t = psp.tile([128, 2, LT], f32, tag="sc", bufs=4,
                                   name="ps_av")
                ps_av = ps_av_t[0:HD + 1, 0, :]
                for mcn in range(L // 128):
                    nc.tensor.matmul(ps_av, v_aug[:, mcn, h, :],
                                     ept_p[:, mcn, i, :],
                                     start=(mcn == 0),
                                     stop=(mcn == L // 128 - 1))
                den = sbp.tile([1, LT], f32, tag="den", bufs=1, name="den")
                nc.vector.tensor_copy(out=den, in_=ps_av[HD:HD + 1, :])
                rcp = sbp.tile([1, LT], f32, tag="rcp", bufs=1, name="rcp")
                nc.vector.reciprocal_approx_fast(rcp, den)
                rcp_bc = sbp.tile([64, LT], f32, tag="rcp_bc", bufs=2,
                                  name="rcp_bc")
                nc.gpsimd.partition_broadcast(rcp_bc, rcp)
                if i == 0:
                    nc.vector.tensor_tensor(attn_sb[0:64, hc_p, lts_p],
                                            ps_av[:HD, :], rcp_bc, ALU.mult)
                else:
                    at = sbp.tile([64, LT], bf16, tag="at", bufs=2, name="at")
                    nc.vector.tensor_tensor(at, ps_av[:HD, :], rcp_bc,
                                            ALU.mult)
                    nc.sync.dma_start(out=attn_sb[64:128, hc_p, lts_p],
                                      in_=at)

        # QK and AV share ONE 4-buffer PSUM tag: per key chunk, the two
        # row-group-packed matmuls land in one [128, 2, LT] tile (heads of
        # the pair side by side), one merged Exp writes the fused ept tile,
        # and the AV accumulators rotate through the same tag.
        for hc in range(H // 2):
            for lt in range(NLT_OWN):
                lts = slice(lt * LT, (lt + 1) * LT)
                ept_t = sbp.tile([128, L // 128, 2, LT], bf16, tag="ept",
                                 bufs=2, name="ept_t")
                for mcn in range(L // 128):
                    psc = psp.tile([128, 2, LT], f32, tag="sc", bufs=4,
                                   name="psc")
                    ms = slice(mcn * 128, (mcn + 1) * 128)
                    tp0 = (0, 0) if PACK_QK else None
                    tp1 = (64, 0) if PACK_QK else None
                    nc.tensor.matmul(
                        psc[:, 0, :], k_sb[0:64, hc, ms],
                        q_sb[0:64, hc, lts],
                        start=True, stop=True, tile_position=tp0)
                    nc.tensor.matmul(
                        psc[:, 1, :], k_sb[64:128, hc, ms],
                        q_sb[64:128, hc, lts],
                        start=True, stop=True, tile_position=tp1)
                    nc.scalar.activation(ept_t[:, mcn, :, :], psc, AF.Exp)
                emit_av(ept_t, hc, lt)
        if "attn" in dbg:
            nc.sync.dma_start(out=dbg["attn"], in_=attn_sb)

    # ====== phase 4/5: proj + residual, LN2 interleaved per token tile =====
    # proj runs lt-outer so LN2 for tile lt can start while proj of tile
    # lt+1 still has PE work, hiding the LN2 chain latency.
    #
    # h2mod and gelu get NO allocation: they alias storage of persistents
    # that are dead by MLP time (v_aug / k_sb / q_sb).  All reuse is safe by
    # engine program order: every fc1 matmul transitively waits on proj,
    # which waits on the last AV matmul, so no K/V/Q/attn reader can still
    # be in flight when the aliased writes land.
    vflat = v_aug.rearrange("p a h c -> p (a h c)")      # [128, 16640]
    kflat = k_sb.rearrange("p c t -> p (c t)")           # [128, 16384]
    qflat = q_sb.rearrange("p c t -> p (c t)")           # [128, 8192]
    h2mod = vflat[:, 0:DC * LOWN].rearrange("p (c t) -> p c t", t=LOWN)

    def gelu_view(mc):                                   # [128, LOWN] bf16
        if mc < 16:
            return kflat[:, mc * LOWN:(mc + 1) * LOWN]
        if mc < 24:
            return qflat[:, (mc - 16) * LOWN:(mc - 15) * LOWN]
        return vflat[:, DC * LOWN + (mc - 24) * LOWN:
                     DC * LOWN + (mc - 23) * LOWN]

    with ExitStack() as ph:
        sbp = ph.enter_context(tc.tile_pool(name="p4_sb", bufs=2))
        psp = ph.enter_context(tc.tile_pool(name="p4_ps", bufs=1, space="PSUM"))
        wpj = sbp.tile([128, DC, D], bf16, tag="wpj", bufs=1, name="wpj")
        for dc in range(DC):
            nc.sync.dma_start(out=wpj[:, dc, :], in_=wproj_r[:, dc, :])
        for lt in range(NLT_OWN):
            t0 = lt * LT
            ps_s = psp.tile([1, LT], f32, tag="st_s", bufs=1, name="ps_s")
            ps_q = psp.tile([1, LT], f32, tag="st_q", bufs=1, name="ps_q")
            for ft in range(DC):
                ps = psp.tile([128, LT], f32, tag="pj", bufs=2, name="ps_pj")
                for dc in range(DC):
                    nc.tensor.matmul(
                        ps, wpj[:, dc, ft * 128:(ft + 1) * 128],
                        attn_sb[:, dc, lt * LT:(lt + 1) * LT],
                        start=(dc == 0), stop=(dc == DC - 1))
                gh = sbp.tile([128, LT], f32, tag="gh", bufs=3, name="gh")
                nc.scalar.activation(gh, ps, AF.Identity,
                                     bias=gmbp[:, ft:ft + 1],
                                     scale=gate_msa[:, ft:ft + 1])
                xo = x_own[:, ft, t0:t0 + LT]
                nc.vector.tensor_tensor(xo, xo, gh, ALU.add)
                # LN2 stats for this feature chunk, right as it finalizes
                xb = sbp.tile([128, LT], bf16, tag="xb", bufs=2, name="xb")
                nc.scalar.activation(xb, xo, AF.Identity)
                nc.tensor.matmul(ps_s, ones_bf, xb,
                                 start=(ft == 0), stop=(ft == DC - 1))
                sq = sbp.tile([128, LT], bf16, tag="sq", bufs=2, name="sq")
                nc.vector.tensor_tensor(sq, xb, xb, ALU.mult)
                nc.tensor.matmul(ps_q, ones_bf, sq,
                                 start=(ft == 0), stop=(ft == DC - 1))
            mean = sbp.tile([1, LT], f32, tag="ln_mean", bufs=1, name="mean")
            msq = sbp.tile([1, LT], f32, tag="ln_msq", bufs=1, name="msq")
            nc.vector.tensor_scalar_mul(mean, ps_s, 1.0 / D)
            nc.vector.tensor_tensor(msq, mean, mean, ALU.mult)
            nc.vector.scalar_tensor_tensor(msq, ps_q, 1.0 / D, msq,
                                           ALU.mult, ALU.subtract)
            sd = sbp.tile([1, LT], f32, tag="ln_sd", bufs=1, name="sd")
            nc.scalar.activation(sd, msq, AF.Sqrt, bias=eps_tile)
            rstd = sbp.tile([1, LT], f32, tag="rstd_s", bufs=1, name="rstd")
            nc.vector.reciprocal_approx_fast(rstd, sd)
            mua = sbp.tile([1, LT], bf16, tag="mua_s", bufs=1, name="mua")
            nc.vector.tensor_tensor(mua, mean, rstd, ALU.mult)
            ln_apply(sbp, x_own[:, :, t0:t0 + LT],
                     h2mod[:, :, t0:t0 + LT], rstd, mua, s_mlp, shift_mlp,
                     sub_on_pool=False)
        if "x1" in dbg:
            nc.sync.dma_start(out=dbg["x1"], in_=x_own)
        if "h2" in dbg:
            nc.sync.dma_start(out=dbg["h2"], in_=h2mod)

    with ExitStack() as ph:
        sbp = ph.enter_context(tc.tile_pool(name="p6_sb", bufs=2))
        psp = ph.enter_context(tc.tile_pool(name="p6_ps", bufs=1, space="PSUM"))
        STAG = 6   # fts of lt0-work emitted before each ft's lt1 chain
        w1tiles = []
        for i in range(MC + STAG):
            if i < MC:
                ft = i
                w1b = sbp.tile([128, DC, 128], bf16, tag="w1b", bufs=STAG + 2,
                               name="w1b")
                nc.sync.dma_start(out=w1b,
                                  in_=w1_r[:, :, ft * 128:(ft + 1) * 128])
                w1tiles.append(w1b)
                ps0 = psp.tile([128, LT], f32, tag="f10", bufs=2,
                               name="ps_f10")
                for dc in range(DC):
                    nc.tensor.matmul(ps0, w1b[:, dc, :], h2mod[:, dc, 0:LT],
                                     start=(dc == 0), stop=(dc == DC - 1))
                nc.scalar.activation(gelu_view(ft)[:, 0:LT], ps0, AF.Gelu,
                                     bias=bias_sb["b1"][:, ft:ft + 1])
            if i >= STAG:
                ft2 = i - STAG
                ps1 = psp.tile([128, LT], f32, tag="f11", bufs=2,
                               name="ps_f11")
                for dc in range(DC):
                    nc.tensor.matmul(ps1, w1tiles[ft2][:, dc, :],
                                     h2mod[:, dc, LT:2 * LT],
                                     start=(dc == 0), stop=(dc == DC - 1))
                nc.scalar.activation(gelu_view(ft2)[:, LT:2 * LT], ps1,
                                     AF.Gelu,
                                     bias=bias_sb["b1"][:, ft2:ft2 + 1])

    outr = out.rearrange("(c p) t -> p c t", p=128)
    with ExitStack() as ph:
        sbp = ph.enter_context(tc.tile_pool(name="p7_sb", bufs=2))
        psp = ph.enter_context(tc.tile_pool(name="p7_ps", bufs=1, space="PSUM"))
        for ft in range(DC):
            w2b = sbp.tile([128, MC, 128], bf16, tag="w2b", bufs=2,
                           name="w2b")
            nc.sync.dma_start(out=w2b,
                              in_=w2_r[:, :, ft * 128:(ft + 1) * 128])
            ps = [psp.tile([128, LT], f32, tag=f"f2{i}", bufs=2,
                           name=f"ps_f2{i}") for i in range(NLT_OWN)]
            for mc in range(MC):
                gv = gelu_view(mc)
                for lt in range(NLT_OWN):
                    nc.tensor.matmul(
                        ps[lt], w2b[:, mc, :],
                        gv[:, lt * LT:(lt + 1) * LT],
                        start=(mc == 0), stop=(mc == MC - 1))
            for lt in range(NLT_OWN):
                gh = sbp.tile([128, LT], f32, tag="gh2", bufs=3, name="gh2")
                nc.scalar.activation(gh, ps[lt], AF.Identity,
                                     bias=gmb2[:, ft:ft + 1],
                                     scale=gate_mlp[:, ft:ft + 1])
                xo = x_own[:, ft, lt * LT:(lt + 1) * LT]
                nc.vector.tensor_tensor(xo, xo, gh, ALU.add)
            nc.sync.dma_start(out=outr[:, ft, :], in_=x_own[:, ft, :])

    # release persistents in reverse creation order
    fr_q()
    fr_v()
    fr_k()
    fr_x_own()
    fr_g2(); fr_g1(); fr_s2(); fr_s1(); fr_tp()
    fr_bv()
    for fr in reversed(bias_frees):
        fr()
    fr_eps(); fr_one1(); fr_ones_bf()


_PROGRAM_CACHE = {}


def _get_program():
    if "nc" not in _PROGRAM_CACHE:
        _PROGRAM_CACHE["nc"] = build_program()
    return _PROGRAM_CACHE["nc"]


def _fm(v):
    """[D] vector -> feature-major [128, D//128] (partition p, chunk c)."""
    return np.ascontiguousarray(np.asarray(v, np.float32).reshape(-1, 128).T)


def make_in_maps(x, time_emb, Wqkv, bqkv, Wproj, bproj, W1, b1, W2, b2, Wt, bt,
                 g1, be1, g2, be2):
    # g1/be1/g2/be2 are identity layernorm params in this module; verify and
    # fold them away.
    assert np.allclose(g1, 1.0) and np.allclose(g2, 1.0)
    assert np.allclose(be1, 0.0) and np.allclose(be2, 0.0)

    x = np.asarray(x, np.float32)
    wqkv_s = np.asarray(Wqkv, np.float32).copy()
    wqkv_s[:, :D] *= 0.125  # fold the attention scale into Q
    shared = {
        "wqkv": wqkv_s.astype(BF),
        "bq": _fm(np.asarray(bqkv[:D]) * 0.125),
        "bk": _fm(bqkv[D:2 * D]),
        "bv": np.ascontiguousarray(np.asarray(bqkv[2 * D:], np.float32)[None, :]),
        "wproj": np.asarray(Wproj, np.float32).astype(BF),
        "bproj": _fm(bproj),
        "w1": np.asarray(W1, np.float32).astype(BF),
        "b1": _fm(b1),
        "w2": np.asarray(W2, np.float32).astype(BF),
        "b2": _fm(b2),
        "wt": np.asarray(Wt, np.float32).astype(BF),
        "bt": _fm(bt),
    }
    in_maps = []
    for c in range(NCORES):
        b, half = c // 2, c % 2
        xb = x[b].T  # [D, L] feature-major
        own = slice(half * LOWN, (half + 1) * LOWN)
        oth = slice((1 - half) * LOWN, (2 - half) * LOWN)
        m = dict(shared)
        m["xfm"] = np.ascontiguousarray(xb[:, own])
        m["xoth_bf"] = np.ascontiguousarray(xb[:, oth]).astype(BF)
        m["temb"] = _fm(time_emb[b])
        in_maps.append(m)
    return in_maps


def assemble_output(results):
    outp = np.empty((B, L, D), np.float32)
    for c in range(NCORES):
        b, half = c // 2, c % 2
        outp[b, half * LOWN:(half + 1) * LOWN, :] = results[c]["out_fm"].T
    return outp


def kernel(x, time_emb, Wqkv, bqkv, Wproj, bproj, W1, b1, W2, b2, Wt, bt,
           g1, be1, g2, be2, trace=False, trace_kwargs=None):
    in_maps = make_in_maps(x, time_emb, Wqkv, bqkv, Wproj, bproj, W1, b1,
                           W2, b2, Wt, bt, g1, be1, g2, be2)
    nc = _get_program()
    res = run_bass_kernel_spmd(nc, in_maps, core_ids=list(range(NCORES)),
                               trace=trace, trace_kwargs=trace_kwargs or {})
    kernel.last_results = res
    return assemble_output(res.results)


# revision 54
# speedup vs baseline: 1.0378x; 1.0378x over previous
"""DiT block (adaLN) Trainium2 kernel, 8-core SPMD, no collectives.

Sharding: core c handles batch b = c//2 and query-token half c%2 (1024 q
tokens).  Each core computes K/V for all 2048 tokens of its batch (the
only duplicated work), so cores never communicate.  The host permutes
each core's token columns so its own 1024 tokens come first (softmax is
invariant to key order), and transposes x to feature-major [D, L] so the
device never transposes anything.

On-device layout is feature-major everywhere: activations live as
[128 partitions, d-chunk, tokens].  LayerNorm stats (per-token = free
dim) are computed with ones-vector matmuls on the tensor engine (bf16)
and broadcast back across partitions on GpSimd.  All GEMM operands are
bf16 (fp32 PSUM accumulation); the residual stream and LN statistics
stay fp32.

Perf changes vs the 1.95ms baseline (measured 1.08ms):
 - all weight DMAs batched into multi-tile band/block transfers (the SP
   queue pays ~565ns per dma_start regardless of size; the baseline's
   1205 single-tile loads cost ~700us of queue serialization).
 - time-modulation matvec is silu-stationary (free-dim-512 matmuls over
   Wt bands) + 48 tiny transpose-matmuls, instead of 384 free-dim-1
   weight-stationary matmuls each with its own 32KB DMA.
 - LN stats in bf16 (fp32 matmuls run at 1/4 PE rate); the other token
   half arrives host-pre-cast to bf16, skipping the Scalar casts; LN apply
   split across DVE (mult), GpSimd (sub) and Scalar (modulate).
 - phase A interleaves LN1, the tp matvec and the Q/K rounds so the PE
   stays fed while the LN chains drain; Q's output buffer doubles as the
   attention output and the MLP's gelu/h2 buffers alias dead K/V/Q
   storage, fitting everything in SBUF without extra allocations.
 - softmax reciprocal on DVE reciprocal_approx_fast (~5x faster than
   reciprocal; the DVE RECIPROCAL op measured 3.3us per 512 tokens).
 - attention output written straight into SBUF (even heads by the DVE,
   odd heads via one 64-partition-offset SBUF->SBUF DMA per tile)
   instead of a DRAM round trip.
"""

import os
import sys
from contextlib import ExitStack

os.environ.setdefault("MYCRO_LOCAL_CACHE", "1")
for _p in ("/opt/trn_rl_repo", "/root/.axon_site/_ro/trn_rl_repo"):
    if os.path.isdir(_p) and _p not in sys.path:
        sys.path.insert(0, _p)

import ml_dtypes
import numpy as np

import concourse.bass as bass
import concourse.tile as tile
from concourse import bacc, mybir
from concourse.bass_utils import run_bass_kernel_spmd

B, L, D, H, HD, MLPD = 4, 2048, 1024, 16, 64, 4096
NCORES = 8
LOWN = L // 2          # own query tokens per core
DC = D // 128          # 8 chunks of the model dim
MC = MLPD // 128       # 32 chunks of the mlp dim
LT = 512               # token tile for matmul free dim
NLT_OWN = LOWN // LT   # 2 token tiles (queries)

f32 = mybir.dt.float32
bf16 = mybir.dt.bfloat16
AF = mybir.ActivationFunctionType
ALU = mybir.AluOpType
BF = ml_dtypes.bfloat16

PACK_QK = True
DEBUG = bool(int(os.environ.get("KERNEL_DEBUG", "0")))


def build_program():
    # Bacc (not plain Bass): its compile() pass legalizes multi-semaphore
    # waits (event semaphores, nop fusion) that walrus can't encode raw.
    nc = bacc.Bacc()

    def _in(name, shape, dtype):
        return nc.declare_dram_parameter(name, shape, dtype, False)[:]

    xfm = _in("xfm", [D, LOWN], f32)
    xoth = _in("xoth_bf", [D, LOWN], bf16)
    temb = _in("temb", [128, DC], f32)
    wqkv = _in("wqkv", [D, 3 * D], bf16)   # Q section pre-scaled by 1/8
    bq = _in("bq", [128, DC], f32)         # pre-scaled by 1/8
    bk = _in("bk", [128, DC], f32)
    bv = _in("bv", [1, D], f32)
    wproj = _in("wproj", [D, D], bf16)
    bproj = _in("bproj", [128, DC], f32)
    w1 = _in("w1", [D, MLPD], bf16)
    b1 = _in("b1", [128, MC], f32)
    w2 = _in("w2", [MLPD, D], bf16)
    b2 = _in("b2", [128, DC], f32)
    wt = _in("wt", [D, 6 * D], bf16)
    bt = _in("bt", [128, 48], f32)
    out = nc.declare_dram_parameter("out_fm", [D, LOWN], f32, True)[:]

    dbg = {}
    if DEBUG:
        dbg["tp"] = nc.declare_dram_parameter("dbg_tp", [128, 48], f32,
                                              True)[:]
        dbg["xmod"] = nc.declare_dram_parameter("dbg_xmod", [128, DC, L],
                                                bf16, True)[:]
        dbg["q"] = nc.declare_dram_parameter("dbg_q", [128, DC, LOWN], bf16,
                                             True)[:]
        dbg["k"] = nc.declare_dram_parameter("dbg_k", [128, DC, L], bf16,
                                             True)[:]
        dbg["v"] = nc.declare_dram_parameter("dbg_v",
                                             [128, L // 128, H, HD + 1],
                                             bf16, True)[:]
        dbg["attn"] = nc.declare_dram_parameter("dbg_attn", [128, DC, LOWN],
                                                bf16, True)[:]
        dbg["x1"] = nc.declare_dram_parameter("dbg_x1", [128, DC, LOWN],
                                              f32, True)[:]
        dbg["h2"] = nc.declare_dram_parameter("dbg_h2", [128, DC, LOWN],
                                              bf16, True)[:]

    with tile.TileContext(nc) as tc:
        _emit_kernel(tc, xfm, xoth, temb, wqkv, bq, bk, bv, wproj, bproj,
                     w1, b1, w2, b2, wt, bt, out, dbg)
    nc.finalize()  # runs Bacc.compile(): reg alloc + sync legalization
    return nc


def _emit_kernel(tc, xfm, xoth, temb, wqkv, bq, bk, bv, wproj, bproj, w1, b1,
                 w2, b2, wt, bt, out, dbg=None):
    nc = tc.nc
    dbg = dbg or {}

    # ---- persistent constants / host-prepped vectors (freed last) ----
    ones_bf, fr_ones_bf = tc.tile([128, 1], bf16, name="ones_bf")
    nc.vector.memset(ones_bf, 1.0)
    one1, fr_one1 = tc.tile([1, 1], bf16, name="one1")
    nc.vector.memset(one1, 1.0)
    eps_tile, fr_eps = tc.tile([1, 1], f32, name="eps_tile")
    nc.vector.memset(eps_tile, 1e-5)

    bias_sb = {}
    bias_frees = []
    for name, ap, w in (("bq", bq, DC), ("bk", bk, DC), ("bproj", bproj, DC),
                        ("b1", b1, MC), ("b2", b2, DC), ("bt", bt, 48),
                        ("temb", temb, DC)):
        t, fr = tc.tile([128, w], f32, name=f"sb_{name}")
        nc.sync.dma_start(out=t, in_=ap)
        bias_sb[name] = t
        bias_frees.append(fr)
    bv_bc, fr_bv = tc.tile([128, D], f32, name="bv_bc")
    nc.sync.dma_start(
        out=bv_bc,
        in_=bass.AP(tensor=bv.tensor, offset=bv.offset,
                    ap=[[0, 128]] + [list(x) for x in bv.ap[1:]]))

    # modulation vectors (computed in phase 0, consumed later)
    tp, fr_tp = tc.tile([128, 48], f32, name="tp")
    s_msa, fr_s1 = tc.tile([128, DC], f32, name="s_msa")
    s_mlp, fr_s2 = tc.tile([128, DC], f32, name="s_mlp")
    gmbp, fr_g1 = tc.tile([128, DC], f32, name="gmbp")
    gmb2, fr_g2 = tc.tile([128, DC], f32, name="gmb2")
    shift_msa = tp[:, 0:8]
    gate_msa = tp[:, 16:24]
    shift_mlp = tp[:, 24:32]
    gate_mlp = tp[:, 40:48]

    # ---- big persistent activations, creation order = reverse free order ----
    x_own, fr_x_own = tc.tile([128, DC, LOWN], f32, name="x_own")
    k_sb, fr_k = tc.tile([128, DC, L], bf16, name="k_sb")
    # v_aug: [token-part, token-chunk, head, 65]; col 64 holds ones so the
    # AV matmul also produces the softmax denominator.
    v_aug, fr_v = tc.tile([128, L // 128, H, HD + 1], bf16, name="v_aug")
    q_sb, fr_q = tc.tile([128, DC, LOWN], bf16, name="q_sb")
    xmod, fr_xmod = tc.tile([128, DC, L], bf16, name="xmod")

    xr = xfm.rearrange("(c p) t -> p c t", p=128)
    nc.sync.dma_start(out=x_own, in_=xr)
    xoth_r = xoth.rearrange("(c p) t -> p c t", p=128)

    wt_r = wt.rearrange("(c p) f -> p c f", p=128)        # [128, 8, 6144]
    wqkv_r = wqkv.rearrange("(c p) f -> p c f", p=128)    # [128, 8, 3072]
    wproj_r = wproj.rearrange("(c p) f -> p c f", p=128)  # [128, 8, 1024]
    w1_r = w1.rearrange("(c p) f -> p c f", p=128)        # [128, 8, 4096]
    w2_r = w2.rearrange("(c p) f -> p c f", p=128)        # [128, 32, 1024]

    # ---- LayerNorm helpers (512-token tiles) ------------------------------
    # Stats: per-token sum / sumsq via ones-stationary bf16 matmuls (the
    # fp32 matmul path runs at 1/4 PE rate).  x_bf is an optional bf16 view
    # of the same data (skips the Scalar casts).  Returns (rstd, mua) bf16
    # row tiles [1, LT].
    def ln_stats(sbp, psp, x_view, tag, x_bf=None):
        ps_s = psp.tile([1, LT], f32, tag="st_s", bufs=1, name="ps_s")
        ps_q = psp.tile([1, LT], f32, tag="st_q", bufs=1, name="ps_q")
        for dc in range(DC):
            if x_bf is not None:
                xb = x_bf[:, dc, :]
            else:
                xb = sbp.tile([128, LT], bf16, tag="xb", bufs=1, name="xb")
                nc.scalar.activation(xb, x_view[:, dc, :], AF.Identity)
            nc.tensor.matmul(ps_s, ones_bf, xb,
                             start=(dc == 0), stop=(dc == DC - 1))
            sq = sbp.tile([128, LT], bf16, tag="sq", bufs=1, name="sq")
            nc.vector.tensor_tensor(sq, xb, xb, ALU.mult)
            nc.tensor.matmul(ps_q, ones_bf, sq,
                             start=(dc == 0), stop=(dc == DC - 1))
        mean = sbp.tile([1, LT], f32, tag="ln_mean", bufs=1, name="mean")
        msq = sbp.tile([1, LT], f32, tag="ln_msq", bufs=1, name="msq")
        nc.vector.tensor_scalar_mul(mean, ps_s, 1.0 / D)
        nc.vector.tensor_tensor(msq, mean, mean, ALU.mult)
        # msq <- ps_q/D - mean^2  (= var)
        nc.vector.scalar_tensor_tensor(msq, ps_q, 1.0 / D, msq,
                                       ALU.mult, ALU.subtract)
        sd = sbp.tile([1, LT], f32, tag="ln_sd", bufs=1, name="sd")
        nc.scalar.activation(sd, msq, AF.Sqrt, bias=eps_tile)
        # reciprocal_approx_fast is a custom DVE op: needs partition-0-based
        # input and must not run in place.
        rstd = sbp.tile([1, LT], f32, tag=f"rstd{tag}", bufs=1, name="rstd")
        nc.vector.reciprocal_approx_fast(rstd, sd)
        mua = sbp.tile([1, LT], bf16, tag=f"mua{tag}", bufs=1, name="mua")
        nc.vector.tensor_tensor(mua, mean, rstd, ALU.mult)
        return rstd, mua

    # Apply: out = ((x*rstd_bc - mua_bc) * s_d + sh_d) with the three
    # elementwise passes split DVE / GpSimd / Scalar.
    def ln_apply(sbp, x_view, out_view, rstd, mua, scale_ap, shift_ap,
                 sub_on_pool=True):
        a_bc = sbp.tile([128, LT], f32, tag="a_bc", bufs=1, name="a_bc")
        nc.gpsimd.partition_broadcast(a_bc, rstd)
        m_bc = sbp.tile([128, LT], bf16, tag="m_bc", bufs=1, name="m_bc")
        nc.gpsimd.partition_broadcast(m_bc, mua)
        for dc in range(DC):
            t = sbp.tile([128, LT], bf16, tag="ln_t", bufs=2, name="ln_t")
            nc.vector.tensor_tensor(t, x_view[:, dc, :], a_bc, ALU.mult)
            if sub_on_pool:
                nc.gpsimd.tensor_tensor(t, t, m_bc, ALU.subtract)
            else:
                nc.vector.tensor_tensor(t, t, m_bc, ALU.subtract)
            nc.scalar.activation(
                out_view[:, dc, :], t, AF.Identity,
                bias=shift_ap[:, dc:dc + 1], scale=scale_ap[:, dc:dc + 1])

    # ===== phase A: LN1 + time modulation + Q/K, interleaved for overlap ===
    # Order: own-tile LN stats (PE work gated only on the x_own DMA) ->
    # tp matvec (gated on Wt DMAs) -> own applies -> Q round (PE work that
    # hides the streamed other-half LN chains) -> other-half LN -> K round.
    nc.vector.memset(v_aug[:, :, :, HD:], 1.0)
    with ExitStack() as ph:
        sbp = ph.enter_context(tc.tile_pool(name="pA_sb", bufs=2))
        psp = ph.enter_context(tc.tile_pool(name="pA_ps", bufs=1,
                                            space="PSUM"))

        # silu first so the Sigmoid->Sqrt act-table switch happens once
        sig = sbp.tile([128, DC], f32, tag="sig", bufs=1, name="sig")
        nc.scalar.activation(sig, bias_sb["temb"], AF.Sigmoid)
        silu_bf = sbp.tile([128, DC], bf16, tag="silu", bufs=1, name="silu_bf")
        nc.vector.tensor_tensor(silu_bf, bias_sb["temb"], sig, ALU.mult)

        stats_own = []
        for i in range(NLT_OWN):
            stats_own.append(ln_stats(
                sbp, psp, x_own[:, :, i * LT:(i + 1) * LT], f"o{i}"))

        # --- tp = silu(temb) @ Wt + bt, feature-major [128, 48] ---
        ps_tp = psp.tile([128, 48], f32, tag="tp", bufs=1, name="ps_tp")
        for g in range(6):           # column sixths of Wt (1024 each)
            pss = [psp.tile([1, LT], f32, tag=f"tpr{i}", bufs=1,
                            name=f"ps_tpr{i}") for i in range(2)]
            for dc in range(DC):
                band = sbp.tile([128, 1024], bf16, tag="wtb", bufs=2,
                                name="wtb")
                nc.sync.dma_start(
                    out=band, in_=wt_r[:, dc, g * 1024:(g + 1) * 1024])
                for i in range(2):
                    nc.tensor.matmul(pss[i], silu_bf[:, dc:dc + 1],
                                     band[:, i * LT:(i + 1) * LT],
                                     start=(dc == 0), stop=(dc == DC - 1))
            for i in range(2):
                # [1, 512] row chunk -> 4 transposed [128, 1] PSUM columns
                row = sbp.tile([1, LT], bf16, tag="tprow", bufs=1,
                               name="tprow")
                nc.scalar.activation(row, pss[i], AF.Identity)
                for j in range(4):
                    f = g * 8 + i * 4 + j
                    nc.tensor.matmul(ps_tp[:, f:f + 1],
                                     row[0:1, j * 128:(j + 1) * 128],
                                     one1, start=True, stop=True)
        nc.vector.tensor_tensor(tp, ps_tp, bias_sb["bt"], ALU.add)
        nc.vector.tensor_scalar_add(s_msa, tp[:, 8:16], 1.0)
        nc.vector.tensor_scalar_add(s_mlp, tp[:, 32:40], 1.0)
        nc.vector.tensor_tensor(gmbp, gate_msa, bias_sb["bproj"], ALU.mult)
        nc.vector.tensor_tensor(gmb2, gate_mlp, bias_sb["b2"], ALU.mult)

        # --- LN1 apply for own tiles ---
        for i in range(NLT_OWN):
            rstd, mua = stats_own[i]
            ln_apply(sbp, x_own[:, :, i * LT:(i + 1) * LT],
                     xmod[:, :, i * LT:(i + 1) * LT],
                     rstd, mua, s_msa, shift_msa)

        # --- Q round: 2 column quarters, weight-stationary ---
        def qk_quarter(base_col, f4s, nlt, out_sb, bias_name):
            w4 = sbp.tile([128, DC, 512], bf16, tag="wqk4", bufs=2,
                          name="w4")
            for dc in range(DC):
                nc.sync.dma_start(
                    out=w4[:, dc, :],
                    in_=wqkv_r[:, dc, base_col:base_col + 512])
            for lt0 in range(0, nlt, 2):
                for f4 in range(4):
                    f8 = f4s + f4
                    nl = min(2, nlt - lt0)
                    ps = [psp.tile([128, LT], f32, tag=f"qk{i}", bufs=1,
                                   name=f"ps_qk{i}") for i in range(nl)]
                    for dc in range(DC):
                        for i in range(nl):
                            lt = lt0 + i
                            nc.tensor.matmul(
                                ps[i], w4[:, dc, f4 * 128:(f4 + 1) * 128],
                                xmod[:, dc, lt * LT:(lt + 1) * LT],
                                start=(dc == 0), stop=(dc == DC - 1))
                    for i in range(nl):
                        lt = lt0 + i
                        nc.scalar.activation(
                            out_sb[:, f8, lt * LT:(lt + 1) * LT], ps[i],
                            AF.Identity,
                            bias=bias_sb[bias_name][:, f8:f8 + 1])

        for quarter in range(2):
            qk_quarter(quarter * 512, quarter * 4, NLT_OWN, q_sb, "bq")

        # --- other token half: streamed bf16 tiles (host pre-cast) ---
        for i in range(NLT_OWN):
            xo = sbp.tile([128, DC, LT], bf16, tag="xoth", bufs=1, name="xo")
            nc.sync.dma_start(
                out=xo, in_=xoth_r[:, :, i * LT:(i + 1) * LT])
            rstd, mua = ln_stats(sbp, psp, xo, "s", x_bf=xo)
            ln_apply(sbp, xo,
                     xmod[:, :, LOWN + i * LT:LOWN + (i + 1) * LT],
                     rstd, mua, s_msa, shift_msa)

        # --- K round ---
        for quarter in range(2):
            qk_quarter(1024 + quarter * 512, quarter * 4, L // LT, k_sb,
                       "bk")

        if "tp" in dbg:
            nc.sync.dma_start(out=dbg["tp"], in_=tp)
        if "xmod" in dbg:
            nc.sync.dma_start(out=dbg["xmod"], in_=xmod)

    # ================= phase B: V (x-stationary, token-major) ==============
    with ExitStack() as ph:
        sbp = ph.enter_context(tc.tile_pool(name="pB_sb", bufs=2))
        psp = ph.enter_context(tc.tile_pool(name="pB_ps", bufs=1, space="PSUM"))
        wv_sb = sbp.tile([128, DC, D], bf16, tag="wv", bufs=1, name="wv_sb")
        for dc in range(DC):
            nc.sync.dma_start(out=wv_sb[:, dc, :],
                              in_=wqkv_r[:, dc, 2 * 1024:3 * 1024])
        for tcn in range(L // 128):
            psv = psp.tile([128, 2, LT], f32, tag="v", bufs=2, name="ps_v")
            for dc in range(DC):
                for vs in range(2):
                    nc.tensor.matmul(
                        psv[:, vs, :],
                        xmod[:, dc, tcn * 128:(tcn + 1) * 128],
                        wv_sb[:, dc, vs * LT:(vs + 1) * LT],
                        start=(dc == 0), stop=(dc == DC - 1))
            for vs in range(2):
                nc.vector.tensor_tensor(
                    v_aug[:, tcn, vs * 8:(vs + 1) * 8, :HD],
                    psv[:, vs, :], bv_bc[:, vs * LT:(vs + 1) * LT], ALU.add)
        if "q" in dbg:
            nc.sync.dma_start(out=dbg["q"], in_=q_sb)
            nc.sync.dma_start(out=dbg["k"], in_=k_sb)
            nc.sync.dma_start(out=dbg["v"], in_=v_aug)
    fr_xmod()

    # ================= phase 3: attention ================
    # attn output reuses q_sb's storage: AV for head-chunk hc only writes
    # q columns after the QK matmuls of hc consumed them (WAR deps tracked
    # per-region by the Tile framework).
    attn_sb = q_sb
    with ExitStack() as ph:
        sbp = ph.enter_context(tc.tile_pool(name="p3_sb", bufs=2))
        psp = ph.enter_context(tc.tile_pool(name="p3_ps", bufs=1, space="PSUM"))

        def emit_av(ept_p, hc_p, lt_p):
            lts_p = slice(lt_p * LT, (lt_p + 1) * LT)
            for i in range(2):
                h = 2 * hc_p + i
                ps_av = psp.tile([HD + 1, LT], f32, tag="av", bufs=2,
                                 name="ps_av")
                for mcn in range(L // 128):
                    nc.tensor.matmul(ps_av, v_aug[:, mcn, h, :],
                                     ept_p[i][:, mcn, :],
                                     start=(mcn == 0),
                                     stop=(mcn == L // 128 - 1))
                den = sbp.tile([1, LT], f32, tag="den", bufs=1, name="den")
                nc.vector.tensor_copy(out=den, in_=ps_av[HD:HD + 1, :])
                rcp = sbp.tile([1, LT], f32, tag="rcp", bufs=1, name="rcp")
                nc.vector.reciprocal_approx_fast(rcp, den)
                rcp_bc = sbp.tile([64, LT], f32, tag="rcp_bc", bufs=2,
                                  name="rcp_bc")
                nc.gpsimd.partition_broadcast(rcp_bc, rcp)
                if i == 0:
                    nc.vector.tensor_tensor(attn_sb[0:64, hc_p, lts_p],
                                            ps_av[:HD, :], rcp_bc, ALU.mult)
                else:
                    at = sbp.tile([64, LT], bf16, tag="at", bufs=2, name="at")
                    nc.vector.tensor_tensor(at, ps_av[:HD, :], rcp_bc,
                                            ALU.mult)
                    nc.sync.dma_start(out=attn_sb[64:128, hc_p, lts_p],
                                      in_=at)

        # Note: deferring emit_av by one head pair (software pipelining) was
        # measured SLOWER (1.95ms vs 1.63ms) — it extends ept lifetimes and
        # stalls the next pair's QK on buffer reuse. Keep AV inline.
        for hc in range(H // 2):
            # heads h0 (k rows 0:64) and h1 (k rows 64:128) of chunk hc run
            # as concurrent row-group-packed matmuls on the PE array.
            for lt in range(NLT_OWN):
                lts = slice(lt * LT, (lt + 1) * LT)
                ept = [sbp.tile([128, L // 128, LT], bf16, tag=f"ept{i}",
                                bufs=2, name=f"ept{i}") for i in range(2)]
                for mg in range(L // 256):  # pairs of key chunks
                    ps0 = psp.tile([128, 2, LT], f32, tag="sc0", bufs=2,
                                   name="ps0")
                    ps1 = psp.tile([128, 2, LT], f32, tag="sc1", bufs=1,
                                   name="ps1")
                    for j in range(2):
                        ms = slice((mg * 2 + j) * 128, (mg * 2 + j + 1) * 128)
                        tp0 = (0, 0) if PACK_QK else None
                        tp1 = (64, 0) if PACK_QK else None
                        nc.tensor.matmul(
                            ps0[:, j, :], k_sb[0:64, hc, ms],
                            q_sb[0:64, hc, lts],
                            start=True, stop=True, tile_position=tp0)
                        nc.tensor.matmul(
                            ps1[:, j, :], k_sb[64:128, hc, ms],
                            q_sb[64:128, hc, lts],
                            start=True, stop=True, tile_position=tp1)
                    nc.scalar.activation(ept[0][:, mg * 2:mg * 2 + 2, :],
                                         ps0, AF.Exp)
                    nc.scalar.activation(ept[1][:, mg * 2:mg * 2 + 2, :],
                                         ps1, AF.Exp)
                emit_av(ept, hc, lt)
        if "attn" in dbg:
            nc.sync.dma_start(out=dbg["attn"], in_=attn_sb)

    # ====== phase 4/5: proj + residual, LN2 interleaved per token tile =====
    # proj runs lt-outer so LN2 for tile lt can start while proj of tile
    # lt+1 still has PE work, hiding the LN2 chain latency.
    #
    # h2mod and gelu get NO allocation: they alias storage of persistents
    # that are dead by MLP time (v_aug / k_sb / q_sb).  All reuse is safe by
    # engine program order: every fc1 matmul transitively waits on proj,
    # which waits on the last AV matmul, so no K/V/Q/attn reader can still
    # be in flight when the aliased writes land.
    vflat = v_aug.rearrange("p a h c -> p (a h c)")      # [128, 16640]
    kflat = k_sb.rearrange("p c t -> p (c t)")           # [128, 16384]
    qflat = q_sb.rearrange("p c t -> p (c t)")           # [128, 8192]
    h2mod = vflat[:, 0:DC * LOWN].rearrange("p (c t) -> p c t", t=LOWN)

    def gelu_view(mc):                                   # [128, LOWN] bf16
        if mc < 16:
            return kflat[:, mc * LOWN:(mc + 1) * LOWN]
        if mc < 24:
            return qflat[:, (mc - 16) * LOWN:(mc - 15) * LOWN]
        return vflat[:, DC * LOWN + (mc - 24) * LOWN:
                     DC * LOWN + (mc - 23) * LOWN]

    with ExitStack() as ph:
        sbp = ph.enter_context(tc.tile_pool(name="p4_sb", bufs=2))
        psp = ph.enter_context(tc.tile_pool(name="p4_ps", bufs=1, space="PSUM"))
        wpj = sbp.tile([128, DC, D], bf16, tag="wpj", bufs=1, name="wpj")
        for dc in range(DC):
            nc.sync.dma_start(out=wpj[:, dc, :], in_=wproj_r[:, dc, :])
        for lt in range(NLT_OWN):
            t0 = lt * LT
            ps_s = psp.tile([1, LT], f32, tag="st_s", bufs=1, name="ps_s")
            ps_q = psp.tile([1, LT], f32, tag="st_q", bufs=1, name="ps_q")
            for ft in range(DC):
                ps = psp.tile([128, LT], f32, tag="pj", bufs=2, name="ps_pj")
                for dc in range(DC):
                    nc.tensor.matmul(
                        ps, wpj[:, dc, ft * 128:(ft + 1) * 128],
                        attn_sb[:, dc, lt * LT:(lt + 1) * LT],
                        start=(dc == 0), stop=(dc == DC - 1))
                gh = sbp.tile([128, LT], f32, tag="gh", bufs=3, name="gh")
                nc.scalar.activation(gh, ps, AF.Identity,
                                     bias=gmbp[:, ft:ft + 1],
                                     scale=gate_msa[:, ft:ft + 1])
                xo = x_own[:, ft, t0:t0 + LT]
                nc.vector.tensor_tensor(xo, xo, gh, ALU.add)
                # LN2 stats for this feature chunk, right as it finalizes
                xb = sbp.tile([128, LT], bf16, tag="xb", bufs=2, name="xb")
                nc.scalar.activation(xb, xo, AF.Identity)
                nc.tensor.matmul(ps_s, ones_bf, xb,
                                 start=(ft == 0), stop=(ft == DC - 1))
                sq = sbp.tile([128, LT], bf16, tag="sq", bufs=2, name="sq")
                nc.vector.tensor_tensor(sq, xb, xb, ALU.mult)
                nc.tensor.matmul(ps_q, ones_bf, sq,
                                 start=(ft == 0), stop=(ft == DC - 1))
            mean = sbp.tile([1, LT], f32, tag="ln_mean", bufs=1, name="mean")
            msq = sbp.tile([1, LT], f32, tag="ln_msq", bufs=1, name="msq")
            nc.vector.tensor_scalar_mul(mean, ps_s, 1.0 / D)
            nc.vector.tensor_tensor(msq, mean, mean, ALU.mult)
            nc.vector.scalar_tensor_tensor(msq, ps_q, 1.0 / D, msq,
                                           ALU.mult, ALU.subtract)
            sd = sbp.tile([1, LT], f32, tag="ln_sd", bufs=1, name="sd")
            nc.scalar.activation(sd, msq, AF.Sqrt, bias=eps_tile)
            rstd = sbp.tile([1, LT], f32, tag="rstd_s", bufs=1, name="rstd")
            nc.vector.reciprocal_approx_fast(rstd, sd)
            mua = sbp.tile([1, LT], bf16, tag="mua_s", bufs=1, name="mua")
            nc.vector.tensor_tensor(mua, mean, rstd, ALU.mult)
            ln_apply(sbp, x_own[:, :, t0:t0 + LT],
                     h2mod[:, :, t0:t0 + LT], rstd, mua, s_mlp, shift_mlp,
                     sub_on_pool=False)
        if "x1" in dbg:
            nc.sync.dma_start(out=dbg["x1"], in_=x_own)
        if "h2" in dbg:
            nc.sync.dma_start(out=dbg["h2"], in_=h2mod)

    with ExitStack() as ph:
        sbp = ph.enter_context(tc.tile_pool(name="p6_sb", bufs=2))
        psp = ph.enter_context(tc.tile_pool(name="p6_ps", bufs=1, space="PSUM"))
        STAG = 6   # fts of lt0-work emitted before each ft's lt1 chain
        w1tiles = []
        for i in range(MC + STAG):
            if i < MC:
                ft = i
                w1b = sbp.tile([128, DC, 128], bf16, tag="w1b", bufs=STAG + 2,
                               name="w1b")
                nc.sync.dma_start(out=w1b,
                                  in_=w1_r[:, :, ft * 128:(ft + 1) * 128])
                w1tiles.append(w1b)
                ps0 = psp.tile([128, LT], f32, tag="f10", bufs=2,
                               name="ps_f10")
                for dc in range(DC):
                    nc.tensor.matmul(ps0, w1b[:, dc, :], h2mod[:, dc, 0:LT],
                                     start=(dc == 0), stop=(dc == DC - 1))
                nc.scalar.activation(gelu_view(ft)[:, 0:LT], ps0, AF.Gelu,
                                     bias=bias_sb["b1"][:, ft:ft + 1])
            if i >= STAG:
                ft2 = i - STAG
                ps1 = psp.tile([128, LT], f32, tag="f11", bufs=2,
                               name="ps_f11")
                for dc in range(DC):
                    nc.tensor.matmul(ps1, w1tiles[ft2][:, dc, :],
                                     h2mod[:, dc, LT:2 * LT],
                                     start=(dc == 0), stop=(dc == DC - 1))
                nc.scalar.activation(gelu_view(ft2)[:, LT:2 * LT], ps1,
                                     AF.Gelu,
                                     bias=bias_sb["b1"][:, ft2:ft2 + 1])

    outr = out.rearrange("(c p) t -> p c t", p=128)
    with ExitStack() as ph:
        sbp = ph.enter_context(tc.tile_pool(name="p7_sb", bufs=2))
        psp = ph.enter_context(tc.tile_pool(name="p7_ps", bufs=1, space="PSUM"))
        for ft in range(DC):
            w2b = sbp.tile([128, MC, 128], bf16, tag="w2b", bufs=2,
                           name="w2b")
            nc.sync.dma_start(out=w2b,
                              in_=w2_r[:, :, ft * 128:(ft + 1) * 128])
            ps = [psp.tile([128, LT], f32, tag=f"f2{i}", bufs=2,
                           name=f"ps_f2{i}") for i in range(NLT_OWN)]
            for mc in range(MC):
                gv = gelu_view(mc)
                for lt in range(NLT_OWN):
                    nc.tensor.matmul(
                        ps[lt], w2b[:, mc, :],
                        gv[:, lt * LT:(lt + 1) * LT],
                        start=(mc == 0), stop=(mc == MC - 1))
            for lt in range(NLT_OWN):
                gh = sbp.tile([128, LT], f32, tag="gh2", bufs=3, name="gh2")
                nc.scalar.activation(gh, ps[lt], AF.Identity,
                                     bias=gmb2[:, ft:ft + 1],
                                     scale=gate_mlp[:, ft:ft + 1])
                xo = x_own[:, ft, lt * LT:(lt + 1) * LT]
                nc.vector.tensor_tensor(xo, xo, gh, ALU.add)
            nc.sync.dma_start(out=outr[:, ft, :], in_=x_own[:, ft, :])

    # release persistents in reverse creation order
    fr_q()
    fr_v()
    fr_k()
    fr_x_own()
    fr_g2(); fr_g1(); fr_s2(); fr_s1(); fr_tp()
    fr_bv()
    for fr in reversed(bias_frees):
        fr()
    fr_eps(); fr_one1(); fr_ones_bf()


_PROGRAM_CACHE = {}


def _get_program():
    if "nc" not in _PROGRAM_CACHE:
        _PROGRAM_CACHE["nc"] = build_program()
    return _PROGRAM_CACHE["nc"]


def _fm(v):
    """[D] vector -> feature-major [128, D//128] (partition p, chunk c)."""
    return np.ascontiguousarray(np.asarray(v, np.float32).reshape(-1, 128).T)


def make_in_maps(x, time_emb, Wqkv, bqkv, Wproj, bproj, W1, b1, W2, b2, Wt, bt,
                 g1, be1, g2, be2):
    # g1/be1/g2/be2 are identity layernorm params in this module; verify and
    # fold them away.
    assert np.allclose(g1, 1.0) and np.allclose(g2, 1.0)
    assert np.allclose(be1, 0.0) and np.allclose(be2, 0.0)

    x = np.asarray(x, np.float32)
    wqkv_s = np.asarray(Wqkv, np.float32).copy()
    wqkv_s[:, :D] *= 0.125  # fold the attention scale into Q
    shared = {
        "wqkv": wqkv_s.astype(BF),
        "bq": _fm(np.asarray(bqkv[:D]) * 0.125),
        "bk": _fm(bqkv[D:2 * D]),
        "bv": np.ascontiguousarray(np.asarray(bqkv[2 * D:], np.float32)[None, :]),
        "wproj": np.asarray(Wproj, np.float32).astype(BF),
        "bproj": _fm(bproj),
        "w1": np.asarray(W1, np.float32).astype(BF),
        "b1": _fm(b1),
        "w2": np.asarray(W2, np.float32).astype(BF),
        "b2": _fm(b2),
        "wt": np.asarray(Wt, np.float32).astype(BF),
        "bt": _fm(bt),
    }
    in_maps = []
    for c in range(NCORES):
        b, half = c // 2, c % 2
        xb = x[b].T  # [D, L] feature-major
        own = slice(half * LOWN, (half + 1) * LOWN)
        oth = slice((1 - half) * LOWN, (2 - half) * LOWN)
        m = dict(shared)
        m["xfm"] = np.ascontiguousarray(xb[:, own])
        m["xoth_bf"] = np.ascontiguousarray(xb[:, oth]).astype(BF)
        m["temb"] = _fm(time_emb[b])
        in_maps.append(m)
    return in_maps


def assemble_output(results):
    outp = np.empty((B, L, D), np.float32)
    for c in range(NCORES):
        b, half = c // 2, c % 2
        outp[b, half * LOWN:(half + 1) * LOWN, :] = results[c]["out_fm"].T
    return outp


def kernel(x, time_emb, Wqkv, bqkv, Wproj, bproj, W1, b1, W2, b2, Wt, bt,
           g1, be1, g2, be2, trace=False, trace_kwargs=None):
    in_maps = make_in_maps(x, time_emb, Wqkv, bqkv, Wproj, bproj, W1, b1,
                           W2, b2, Wt, bt, g1, be1, g2, be2)
    nc = _get_program()
    res = run_bass_kernel_spmd(nc, in_maps, core_ids=list(range(NCORES)),
                               trace=trace, trace_kwargs=trace_kwargs or {})
    kernel.last_results = res
    return assemble_output(res.results)
